# revision 1
# baseline (speedup 1.0000x reference)
"""Trainium2 kernel for nn_Dense_Q_MulIn1Out_Conv1D.

The reference "quantum conv" circuit is linear in the state vector: three
RY-rotation layers interleaved with a fixed 512x512 orthogonal entangler.
The whole circuit therefore collapses to one matrix M (512x512), and since
the encoded state has only its first 128 amplitudes nonzero, the <Z> readout
reduces to a quadratic form with a fixed symmetric 128x128 matrix A:

    out[n] = (v_n^T A v_n) / (||v_n||^2 + 1e-12)

where v_n is the (unnormalized) im2col patch of x (C=16 channels x K=8 taps,
channel-major).  A = Md^T Z Md with Md = M[:, :128], Z = diag(+1 x256, -1 x256).

Host side: build A (float64) from entangle_matrix/theta, permute it to
k-major patch order so the on-device im2col is 8 shifted row-block copies.
Device side (per core, 2 of 16 batches): build V [128, 4096] per batch by
DMA, Y = A @ V on TensorE (fp32r), P1 = V*Y, P2 = V*V elementwise, then
ones-vector matmuls reduce partitions to num/den rows of a [16, 512] PSUM
tile; final reciprocal-multiply and one 32KB store.
"""

import numpy as np

_DIM = 512
_D = 128
_K = 8
_C = 16
_NQ = 9
_B = 16
_L = 4096
_L_OUT = _L - _K + 1  # 4089
_N_CORES = 8
_B_PER_CORE = _B // _N_CORES  # 2
_NCHUNK = 8  # 512-column chunks per batch
_CHUNK = 512

# k-major patch permutation: new index p = k*16 + c  <->  old index c*8 + k
_PERM = np.array([(p % _C) * _K + (p // _C) for p in range(_D)])


def _apply_ry_layer(psi, angles):
    # psi [N, DIM] float64; matches reference._apply_ry_layer
    for q in range(_NQ):
        half = angles[q] * 0.5
        c, s = np.cos(half), np.sin(half)
        left = 2 ** q
        p = psi.reshape(-1, left, 2, _DIM // (2 ** (q + 1)))
        a, b = p[:, :, 0, :].copy(), p[:, :, 1, :].copy()
        psi = np.stack([c * a - s * b, s * a + c * b], axis=2).reshape(-1, _DIM)
    return psi


def _build_amat(entangle_matrix, theta):
    """Collapse the circuit to the k-major-permuted 128x128 quadratic form."""
    U = np.asarray(entangle_matrix, dtype=np.float64)
    th = np.asarray(theta, dtype=np.float64)
    psi = np.eye(_DIM, dtype=np.float64)
    for l in range(th.shape[0]):
        psi = _apply_ry_layer(psi, th[l])
        psi = psi @ U.T
    M = psi.T  # state map: s -> M s
    z = np.concatenate([np.ones(_DIM // 2), -np.ones(_DIM // 2)])
    Md = M[:, :_D]
    A = Md.T @ (z[:, None] * Md)
    A_km = A[np.ix_(_PERM, _PERM)]
    return np.ascontiguousarray(A_km, dtype=np.float32)


_NC_CACHE = {}


def _build_nc():
    import concourse.tile as tile
    from concourse import bacc, mybir

    F32 = mybir.dt.float32
    F32R = mybir.dt.float32r
    AF = mybir.ActivationFunctionType

    nc = bacc.Bacc(
        "TRN2",
        target_bir_lowering=False,
        debug=False,
        num_devices=_N_CORES,
    )
    ngl = _B_PER_CORE * _NCHUNK  # 16 global chunks
    # flat x + 8 pad elements so the im2col window never reads out of bounds
    x = nc.dram_tensor(
        "x", [_B_PER_CORE * _C * _L + _K], F32, kind="ExternalInput"
    ).ap()
    # consts = [A_km | T2] with T2 [128, 64]: single ones-column at col 32.
    # A 32-wide window T2[:, 32-m : 64-m] is a selector matrix whose matmul
    # sums all partitions into output partition m (ones at in-window col m).
    consts = nc.dram_tensor(
        "consts", [_D, _D + 96], F32, kind="ExternalInput"
    ).ap()
    out = nc.dram_tensor(
        "out", [_B_PER_CORE * _NCHUNK, _CHUNK], F32, kind="ExternalOutput"
    ).ap()

    with tile.TileContext(nc) as tc:
        from contextlib import ExitStack

        with ExitStack() as ctx:
            const_pool = ctx.enter_context(tc.tile_pool(name="const", bufs=1))
            v_pool = ctx.enter_context(tc.tile_pool(name="v", bufs=2))
            p_pool = ctx.enter_context(tc.tile_pool(name="p", bufs=2))
            y_pool = ctx.enter_context(tc.tile_pool(name="y", bufs=2, space="PSUM"))
            red_pool = ctx.enter_context(tc.tile_pool(name="red", bufs=1, space="PSUM"))
            o_pool = ctx.enter_context(tc.tile_pool(name="o", bufs=1))

            c_sb = const_pool.tile([_D, _D + 96], F32)
            nc.scalar.dma_start(c_sb[:].bitcast(F32R), consts[:].bitcast(F32R))
            a_sb = c_sb[:, :_D]
            t2 = c_sb[:, _D:]

            def sel_num(g):
                # ones at within-window col g -> output partition g (num)
                return t2[:, 48 - g : 96 - g].bitcast(F32R)

            def sel_den(g):
                # ones at col 32+g -> output partition 32+g (den; 32-aligned
                # so the epilogue's partition-offset reads are legal)
                return t2[:, 16 - g : 64 - g].bitcast(F32R)

            # num rows 0..15, den rows 32..47, one PSUM bank total
            red = red_pool.tile([48, _CHUNK], F32)

            from bass_rust import AP as RawAP

            # V free size is _L+1 so its partition pitch (4097) can't be
            # coalesced with the 4096-element column runs by the DMA AP
            # balancer (a flat run crossing SBUF partitions is invalid).
            _LV = _L + 1
            _Q = 1024  # quarter width: DMA piece + y-tile width
            vs = []
            for b in range(_B_PER_CORE):
                v = v_pool.tile([_D, _LV], F32, tag="v")
                vs.append(v)
                # im2col in 4 column-quarters, alternating the two HWDGE
                # rings (sync / scalar) so all 16 SDMA engines run.
                # dst partition (k*16+c), col n <- x[b, c, n+k]; cols >=
                # L_OUT pick up neighboring-channel garbage (host discards).
                for q in range(4):
                    dst = v[:, q * _Q : (q + 1) * _Q].bitcast(F32R)
                    srcap = RawAP(
                        tensor=x.tensor, offset=b * _C * _L + q * _Q,
                        ap=[[1, _K], [_L, _C], [1, _Q]],
                    ).bitcast(F32R)
                    eng = nc.sync if q % 2 == 0 else nc.scalar
                    eng.dma_start(dst, srcap)

            mm_i = 0  # running index over all 64 reduction matmuls
            for b in range(_B_PER_CORE):
                v = vs[b]
                for h in range(2):  # 2048-wide halves for the squares
                    p2 = p_pool.tile([_D, 2 * _Q], F32, tag="p2")
                    nc.scalar.activation(
                        p2[:].bitcast(F32R),
                        v[:, h * 2 * _Q : (h + 1) * 2 * _Q],
                        AF.Square,
                    )
                    for qq in range(2):  # 1024-wide y tiles
                        base = h * 2 * _Q + qq * _Q
                        g0 = b * _NCHUNK + (base // _CHUNK)
                        y = y_pool.tile([_D, _Q], F32)
                        for s in range(2):
                            nc.tensor.matmul(
                                y[:, s * _CHUNK : (s + 1) * _CHUNK],
                                a_sb.bitcast(F32R),
                                v[:, base + s * _CHUNK : base + (s + 1) * _CHUNK]
                                .bitcast(F32R),
                                start=True, stop=True,
                            )
                        p1 = p_pool.tile([_D, _Q], F32, tag="p1")
                        nc.vector.tensor_mul(
                            p1[:].bitcast(F32R), v[:, base : base + _Q], y[:]
                        )
                        for s in range(2):
                            g = g0 + s
                            sl = slice(s * _CHUNK, (s + 1) * _CHUNK)
                            nc.tensor.matmul(
                                red[:], sel_num(g), p1[:, sl].bitcast(F32R),
                                start=(mm_i == 0), stop=(mm_i == 63),
                                skip_group_check=True,
                            )
                            mm_i += 1
                            sl2 = slice(qq * _Q + s * _CHUNK,
                                        qq * _Q + (s + 1) * _CHUNK)
                            nc.tensor.matmul(
                                red[:], sel_den(g), p2[:, sl2].bitcast(F32R),
                                start=(mm_i == 0), stop=(mm_i == 63),
                                skip_group_check=True,
                            )
                            mm_i += 1

            den_sb = o_pool.tile([16, _CHUNK], F32, tag="den")
            nc.scalar.activation(den_sb[:], red[32:48, :], AF.Copy, bias=1e-12)
            rden = o_pool.tile([16, _CHUNK], F32, tag="rden")
            nc.vector.reciprocal_approx_fast(rden[:], den_sb[:])
            out_sb = o_pool.tile([16, _CHUNK], F32, tag="outsb")
            nc.vector.tensor_mul(out_sb[:], red[0:16, :], rden[:])
            nc.sync.dma_start(out[:], out_sb[:])

    nc.compile()
    return nc


def get_nc():
    if "nc" not in _NC_CACHE:
        _NC_CACHE["nc"] = _build_nc()
    return _NC_CACHE["nc"]


def kernel(x, entangle_matrix, theta, _trace=False, **trace_kwargs):
    from concourse.bass_utils import run_bass_kernel_spmd

    x = np.asarray(x, dtype=np.float32)
    amat = _build_amat(entangle_matrix, theta)
    # T2: single ones-column at col 32; sliding 32-wide windows of T2 give
    # every selector matrix (ones exactly at within-block column g).
    t2 = np.zeros((_D, 96), dtype=np.float32)
    t2[:, 48] = 1.0
    consts = np.ascontiguousarray(np.concatenate([amat, t2], axis=1))

    nc = get_nc()
    pad = np.zeros(_K, dtype=np.float32)
    in_maps = [
        {
            "x": np.concatenate(
                [x[i * _B_PER_CORE : (i + 1) * _B_PER_CORE].reshape(-1), pad]
            ),
            "consts": consts,
        }
        for i in range(_N_CORES)
    ]
    res = run_bass_kernel_spmd(
        nc, in_maps, list(range(_N_CORES)), trace=_trace, **trace_kwargs
    )
    outs = []
    for i in range(_N_CORES):
        o = np.asarray(res.results[i]["out"], dtype=np.float32)
        outs.append(o.reshape(_B_PER_CORE, _NCHUNK * _CHUNK)[:, :_L_OUT])
    full = np.concatenate(outs, axis=0).reshape(_B, 1, 1, _L_OUT)
    if _trace:
        kernel._last_results = res
    return full



# revision 5
# speedup vs baseline: 1.3493x; 1.3493x over previous
"""Trainium2 kernel for nn_Dense_Q_MulIn1Out_Conv1D.

The reference "quantum conv" circuit is linear in the state vector: three
RY-rotation layers interleaved with a fixed 512x512 orthogonal entangler.
The whole circuit collapses to one matrix, and since the encoded state has
only its first 128 amplitudes nonzero, the <Z> readout reduces to a
quadratic form with a fixed symmetric 128x128 matrix A:

    out[n] = (v_n^T A v_n) / (||v_n||^2 + 1e-12)

where v_n is the (unnormalized) im2col patch of x (C=16 channels x K=8
taps, channel-major).  A = Md^T Z Md with Md = M[:, :128],
Z = diag(+1 x256, -1 x256); permuted here to k-major patch order so the
on-device im2col is 8 shifted row-block copies.

v2 layout (per core, 2 of 16 batches):
  - x is pre-converted to bf16 on the host; the im2col DMA reads half the
    bytes of the fp32 version (2.1 MB -> 1.05 MB HBM per core).
  - V is built as 8 independent [128, 1024] bf16 stripes (one DMA each) so
    compute pipelines against the DMA at stripe granularity.
  - num: per stripe, Y = A_bf16 @ V (2x 512-col bf16 matmuls into one
    2-bank PSUM tile), p1 = V*Y on DVE (fp32), then two selector matmuls
    accumulate column-sums into the num rows of a [16, 512] PSUM grid.
  - den: ||v_n||^2 = sliding-8 window sum of s[n] = sum_c x[c,n]^2, so it
    is computed from X^2 [32, 4104] directly (130k squares instead of 1M):
    8+8 selector matmuls reduce channels into an s-grid [16, 512+8], and a
    log-tree of 3 shifted DVE adds forms the window sums.  This runs first
    and warms the PE while the V stripes stream in.
"""

import numpy as np

_DIM = 512
_D = 128
_K = 8
_C = 16
_NQ = 9
_B = 16
_L = 4096
_L_OUT = _L - _K + 1  # 4089
_N_CORES = 8
_B_PER_CORE = _B // _N_CORES  # 2
_CHUNK = 512

# k-major patch permutation: new index p = k*16 + c  <->  old index c*8 + k
_PERM = np.array([(p % _C) * _K + (p // _C) for p in range(_D)])


def _apply_ry_layer(psi, angles):
    # psi [N, DIM] float64; matches reference._apply_ry_layer
    for q in range(_NQ):
        half = angles[q] * 0.5
        c, s = np.cos(half), np.sin(half)
        left = 2 ** q
        p = psi.reshape(-1, left, 2, _DIM // (2 ** (q + 1)))
        a, b = p[:, :, 0, :].copy(), p[:, :, 1, :].copy()
        psi = np.stack([c * a - s * b, s * a + c * b], axis=2).reshape(-1, _DIM)
    return psi


def _build_amat(entangle_matrix, theta):
    """Collapse the circuit to the k-major-permuted 128x128 quadratic form."""
    U = np.asarray(entangle_matrix, dtype=np.float64)
    th = np.asarray(theta, dtype=np.float64)
    psi = np.eye(_DIM, dtype=np.float64)
    for l in range(th.shape[0]):
        psi = _apply_ry_layer(psi, th[l])
        psi = psi @ U.T
    M = psi.T  # state map: s -> M s
    z = np.concatenate([np.ones(_DIM // 2), -np.ones(_DIM // 2)])
    Md = M[:, :_D]
    A = Md.T @ (z[:, None] * Md)
    A_km = A[np.ix_(_PERM, _PERM)]
    return np.ascontiguousarray(A_km, dtype=np.float32)


_NC_CACHE = {}


def _build_nc():
    import concourse.tile as tile
    from concourse import bacc, mybir

    F32 = mybir.dt.float32
    F32R = mybir.dt.float32r
    BF16 = mybir.dt.bfloat16
    AF = mybir.ActivationFunctionType

    nc = bacc.Bacc(
        "TRN2",
        target_bir_lowering=False,
        debug=False,
        num_devices=_N_CORES,
    )
    # flat bf16 x + 8 pad elements so the im2col window never reads OOB
    xb = nc.dram_tensor(
        "xb", [_B_PER_CORE * _C * _L + _K], BF16, kind="ExternalInput"
    ).ap()
    # A (k-major, bf16) for the Y matmuls
    cbf = nc.dram_tensor("cbf", [_D, _D], BF16, kind="ExternalInput").ap()
    # fp32 selector table: cols 0..30 = T2 (num: ones at col 15, all rows);
    # cols 31..61 = T2d (den: ones at col 46 rows 0-15, col 54 rows 16-31).
    # A 16-wide window T2[:, 15-g:31-g] is a selector whose matmul sums all
    # 128 partitions into output partition g; T2d[:, 46-g:62-g] sums
    # partitions 0-15 -> row g and 16-31 -> row 8+g.
    cf = nc.dram_tensor("cf", [_D, 62], F32, kind="ExternalInput").ap()
    out = nc.dram_tensor(
        "out", [_B_PER_CORE * _K, _CHUNK], F32, kind="ExternalOutput"
    ).ap()

    _Q = 1024  # stripe width
    _XW = 4104  # xq width: L + 8 halo cols

    with tile.TileContext(nc) as tc:
        from contextlib import ExitStack
        from bass_rust import AP as RawAP

        with ExitStack() as ctx:
            const_pool = ctx.enter_context(tc.tile_pool(name="const", bufs=1))
            x_pool = ctx.enter_context(tc.tile_pool(name="x", bufs=1))
            v_pool = ctx.enter_context(tc.tile_pool(name="v", bufs=8))
            p_pool = ctx.enter_context(tc.tile_pool(name="p", bufs=2))
            y_pool = ctx.enter_context(tc.tile_pool(name="y", bufs=2, space="PSUM"))
            r_pool = ctx.enter_context(tc.tile_pool(name="r", bufs=1, space="PSUM"))
            o_pool = ctx.enter_context(tc.tile_pool(name="o", bufs=1))

            a_sb = const_pool.tile([_D, _D], BF16, tag="a")
            cf_sb = const_pool.tile([_D, 62], F32, tag="cf")
            nc.scalar.dma_start(cf_sb[:].bitcast(F32R), cf[:].bitcast(F32R))
            nc.scalar.dma_start(a_sb[:], cbf[:])

            def sel_num(g):
                return cf_sb[:, 15 - g : 31 - g].bitcast(F32R)

            def sel_den(g):
                return cf_sb[0:32, 46 - g : 62 - g].bitcast(F32R)

            # xq: both batches' channels on partitions 0-31 (+1 col pad so
            # the DMA AP balancer can't fuse partition-crossing runs)
            xq = x_pool.tile([2 * _C, _XW + 1], BF16, tag="xq")
            nc.sync.dma_start(
                xq[:, 0:_XW],
                RawAP(tensor=xb.tensor, offset=0, ap=[[_L, 2 * _C], [1, _XW]]),
            )

            # im2col stripes: V[k*16+c, n] = x[b, c, n+k], 8 stripes of 1024
            vs = []
            for b in range(_B_PER_CORE):
                for q in range(4):
                    v = v_pool.tile([_D, _Q + 1], BF16, tag="v")
                    vs.append(v)
                    srcap = RawAP(
                        tensor=xb.tensor,
                        offset=b * _C * _L + q * _Q,
                        ap=[[1, _K], [_L, _C], [1, _Q]],
                    )
                    eng = nc.sync if (b * 4 + q) % 2 == 0 else nc.scalar
                    eng.dma_start(v[:, 0:_Q], srcap)

            # ---- den path (runs while stripes stream; warms the PE) ----
            x2 = x_pool.tile([2 * _C, _XW], F32, tag="x2")
            half = 2052
            nc.scalar.activation(
                x2[:, 0:half].bitcast(F32R), xq[:, 0:half], AF.Square
            )
            nc.vector.tensor_mul(
                x2[:, half:_XW].bitcast(F32R), xq[:, half:_XW], xq[:, half:_XW]
            )

            s_main = r_pool.tile([16, _CHUNK], F32, tag="smain")
            s_halo = r_pool.tile([16, _K], F32, tag="shalo")
            for g in range(8):
                nc.tensor.matmul(
                    s_main[:],
                    sel_den(g),
                    x2[:, g * _CHUNK : (g + 1) * _CHUNK].bitcast(F32R),
                    start=(g == 0),
                    stop=(g == 7),
                    skip_group_check=True,
                )
            for g in range(8):
                nc.tensor.matmul(
                    s_halo[:],
                    sel_den(g),
                    x2[:, (g + 1) * _CHUNK : (g + 1) * _CHUNK + _K].bitcast(F32R),
                    start=(g == 0),
                    stop=(g == 7),
                    skip_group_check=True,
                )

            s_sb = o_pool.tile([16, _CHUNK + _K], F32, tag="ssb")
            nc.scalar.activation(s_sb[:, 0:_CHUNK], s_main[:], AF.Copy)
            nc.scalar.activation(s_sb[:, _CHUNK : _CHUNK + _K], s_halo[:], AF.Copy)
            # sliding 8-window sum via log tree: 1,2,4-shifted adds
            t1 = o_pool.tile([16, 519], F32, tag="t1")
            nc.vector.tensor_add(t1[:], s_sb[:, 0:519], s_sb[:, 1:520])
            t2 = o_pool.tile([16, 517], F32, tag="t2")
            nc.vector.tensor_add(t2[:], t1[:, 0:517], t1[:, 2:519])
            den_sb = o_pool.tile([16, _CHUNK], F32, tag="den")
            nc.vector.tensor_add(den_sb[:], t2[:, 0:_CHUNK], t2[:, 4 : 4 + _CHUNK])
            denb = o_pool.tile([16, _CHUNK], F32, tag="denb")
            nc.scalar.activation(denb[:], den_sb[:], AF.Copy, bias=1e-12)
            rden = o_pool.tile([16, _CHUNK], F32, tag="rden")
            nc.vector.reciprocal_approx_fast(rden[:], denb[:])

            # ---- num path ----
            red = r_pool.tile([16, _CHUNK], F32, tag="red")
            mm = 0
            for b in range(_B_PER_CORE):
                for q in range(4):
                    v = vs[b * 4 + q]
                    y = y_pool.tile([_D, _Q], F32)
                    for s in range(2):
                        nc.tensor.matmul(
                            y[:, s * _CHUNK : (s + 1) * _CHUNK],
                            a_sb[:],
                            v[:, s * _CHUNK : (s + 1) * _CHUNK],
                            start=True,
                            stop=True,
                        )
                    p1 = p_pool.tile([_D, _Q], F32, tag="p1")
                    nc.vector.tensor_mul(p1[:].bitcast(F32R), v[:, 0:_Q], y[:])
                    for s in range(2):
                        g = b * 8 + q * 2 + s
                        nc.tensor.matmul(
                            red[:],
                            sel_num(g),
                            p1[:, s * _CHUNK : (s + 1) * _CHUNK].bitcast(F32R),
                            start=(mm == 0),
                            stop=(mm == 15),
                            skip_group_check=True,
                        )
                        mm += 1

            out_sb = o_pool.tile([16, _CHUNK], F32, tag="outsb")
            nc.vector.tensor_mul(out_sb[:], red[:], rden[:])
            nc.sync.dma_start(out[:], out_sb[:])

    nc.compile()
    return nc


def get_nc():
    if "nc" not in _NC_CACHE:
        _NC_CACHE["nc"] = _build_nc()
    return _NC_CACHE["nc"]


def kernel(x, entangle_matrix, theta, _trace=False, **trace_kwargs):
    import ml_dtypes
    from concourse.bass_utils import run_bass_kernel_spmd

    bf16 = ml_dtypes.bfloat16
    x = np.asarray(x, dtype=np.float32)
    amat = _build_amat(entangle_matrix, theta)
    a_bf = np.ascontiguousarray(amat.astype(bf16))
    cf = np.zeros((_D, 62), dtype=np.float32)
    cf[:, 15] = 1.0      # num selector ones column
    cf[0:16, 46] = 1.0   # den selector, batch 0 rows
    cf[16:32, 54] = 1.0  # den selector, batch 1 rows

    nc = get_nc()
    pad = np.zeros(_K, dtype=np.float32)
    in_maps = []
    for i in range(_N_CORES):
        xi = np.concatenate(
            [x[i * _B_PER_CORE : (i + 1) * _B_PER_CORE].reshape(-1), pad]
        )
        in_maps.append(
            {"xb": xi.astype(bf16), "cbf": a_bf, "cf": cf}
        )
    res = run_bass_kernel_spmd(
        nc, in_maps, list(range(_N_CORES)), trace=_trace, **trace_kwargs
    )
    outs = []
    for i in range(_N_CORES):
        o = np.asarray(res.results[i]["out"], dtype=np.float32)
        outs.append(o.reshape(_B_PER_CORE, _K * _CHUNK)[:, :_L_OUT])
    full = np.concatenate(outs, axis=0).reshape(_B, 1, 1, _L_OUT)
    if _trace:
        kernel._last_results = res
    return full
